# revision 1
# baseline (speedup 1.0000x reference)
"""Trainium2 Bass kernel for batched attention:
    out[b] = softmax(q[b] @ m[b].T / 0.02) @ m[b]
with q, m: [8, 2048, 1024] fp32.

Sharding: data-parallel over batch — core b computes batch element b.

Numerics: the softmax temperature (x50) makes logits huge (std ~1600), so
softmax is near-one-hot and the scores matmul needs ~fp32 precision to keep
the argmax/mixing stable. Native fp32 matmul costs 4 PE-cycles/row; instead
mm1 uses an fp16 hi/lo 3-pass split (qh*mh + qh*ml + ql*mh, fp32 PSUM
accumulation) at 3 cycles/row with ~22-bit effective mantissa — measured
absmax relative error ~1e-3 vs fp64, same envelope as a pure-fp32 pipeline.
mm2 (P @ M) is plain fp16: P's dominant weights are exactly representable
and fp16(M)'s 11-bit mantissa matches what the hardware's tf32 mode would
keep anyway.

Per-core dataflow (Lq=Lkv=2048, D=1024, q-tile = 128 rows):
  setup:  mh_nat [128,16,1024] f16  = fp16(M) by k-chunk  (mm2 rhs)
          MhT    [128,8,2048]  f16  = fp16(M)^T by d-chunk  (PE transposes)
          MlT    [128,8,2048]  f16  = (M - fp16(M))^T
  per q-tile:
          qh/ql  [128,8,128]   f16  = hi/lo of 50*Q_tile, transposed
          S      = 3-pass matmul -> PSUM [128, 4x512] f32
          P      = exp(S - rowmax) -> SBUF f16 (ACT; accum_out = row sums)
          PT     [128,16,128]  f16  = P^T (PE transposes)
          O      = PT.T @ mh_nat -> PSUM [128,1024] f32
          out    = O * (1/rowsum) -> f32 -> DMA out
"""

import sys

if "/opt/trn_rl_repo" not in sys.path:
    sys.path.insert(0, "/opt/trn_rl_repo")

import os

import numpy as np

# Wait-split carrier opcode: "Drain" (safe: waits + pipe-flush) or "NoOp"
# (waits only). Both compile; NoOp avoids flushing the PE matmul pipe at
# the ~110 split points on hot paths.
SPLIT_OPCODE = os.environ.get("ATTN_SPLIT_OPCODE", "Drain")
# Use DMA xbar transposes (2-byte path) for the Q hi/lo tiles instead of
# PE transposes + engine copies.
QT_DMA_T = os.environ.get("ATTN_QT_DMA_T", "0") == "1"
# mm1 bank pairing: reuse each loaded stationary operand for 2 PSUM banks
# (halves LDWEIGHTS traffic, delays half the bank reduces to mm1 end).
MM1_PAIRED = os.environ.get("ATTN_MM1_PAIRED", "0") == "1"

B = 8
LQ = 2048
LKV = 2048
D = 1024
P = 128
NQT = LQ // P       # 16 q tiles
NKC = LKV // P      # 16 k chunks
NDC = D // P        # 8 d chunks
NS1 = LKV // 512    # 4 n-slices for mm1 (one PSUM bank each)
NS2 = D // 512      # 2 n-slices for mm2
SCALE = 1.0 / 0.02  # 50.0

_CACHE = {}


def _patch_json(nc):
    """This container's walrus supports only ONE sync-wait per instruction.
    Split any multi-wait instruction into preceding single-wait Drains on
    the same engine (engines execute in order, so semantics are identical)."""
    import orjson

    orig = nc.to_json_bytes

    def fixed():
        d = orjson.loads(orig())
        for fn in d["functions"]:
            for bb in fn["blocks"]:
                new = []
                for inst in bb.get("instructions", []):
                    si = inst.get("sync_info") or {}
                    ow = si.get("on_wait") or []
                    if len(ow) > 1:
                        excess, keep = ow[:-1], ow[-1:]
                        si["on_wait"] = keep
                        for k, w in enumerate(excess):
                            new.append({
                                "debug": inst.get("debug", 0),
                                "engine": inst["engine"],
                                "ins": [], "outs": [],
                                "is_reset_sema": False,
                                "name": f"{inst['name']}-sw{k}",
                                "opcode": SPLIT_OPCODE,
                                "sync_info": {"on_update": [], "on_wait": [w]},
                            })
                    new.append(inst)
                bb["instructions"] = new
        return orjson.dumps(d)

    nc.to_json_bytes = fixed
    return nc


def build_nc(loop_r=None):
    """loop_r: when set, wrap the main q-tile loop in a hardware For_i that
    repeats it loop_r times — used only for device-time measurement (wall
    clock through the axon tunnel is transfer-dominated)."""
    import contextlib

    import concourse.bass as bass
    import concourse.mybir as mybir
    import concourse.tile as tile
    from concourse.masks import make_identity

    f32 = mybir.dt.float32
    f16 = mybir.dt.float16
    AX = mybir.AxisListType.X
    EXP = mybir.ActivationFunctionType.Exp

    nc = bass.Bass()
    q_d = nc.dram_tensor("q", [LQ, D], f32, kind="ExternalInput")
    m_d = nc.dram_tensor("m", [LKV, D], f32, kind="ExternalInput")
    o_d = nc.dram_tensor("out", [LQ, D], f32, kind="ExternalOutput")

    q_ap = q_d.ap()
    m_ap = m_d.ap()
    o_ap = o_d.ap()

    with tile.TileContext(nc) as tc:
        with (
            tc.tile_pool(name="const", bufs=1) as const_pool,
            tc.tile_pool(name="mres", bufs=1) as mres_pool,
            tc.tile_pool(name="qload", bufs=3) as qload_pool,
            tc.tile_pool(name="qsplit", bufs=2) as qsplit_pool,
            tc.tile_pool(name="qt", bufs=2) as qt_pool,
            tc.tile_pool(name="psb", bufs=2) as p_pool,
            tc.tile_pool(name="ptt", bufs=2) as pt_pool,
            tc.tile_pool(name="osb", bufs=3) as out_pool,
            tc.tile_pool(name="vec", bufs=6) as vec_pool,
            tc.tile_pool(name="msplit", bufs=3) as msplit_pool,
            tc.tile_pool(name="ps_s", bufs=1, space="PSUM") as ps_s,
            tc.tile_pool(name="ps_o", bufs=1, space="PSUM") as ps_o,
            tc.tile_pool(name="ps_t", bufs=2, space="PSUM") as ps_t,
        ):
            ident16 = const_pool.tile([P, P], f16)
            make_identity(nc, ident16)

            # ---- resident M derivatives: mh_nat (f16, natural), MhT/MlT
            # (f16, transposed by d-chunk).
            # Transposes land in grouped [128, 4x128] PSUM tiles so ONE
            # [128,512] copy moves four transposed blocks to SBUF.
            # Note: a transposed [d, k] block of chunk kc for d-chunk dc sits
            # at mht[:, dc, kc*128:(kc+1)*128] — the four blocks of one group
            # share kc but differ in dc, so group copies go per-(kc, dc-quad):
            # dest mht[:, dc0:dc0+4, kc...] is NOT contiguous. Instead group
            # four k-chunks? they differ in kc → dest [128, dc, 4*128] IS
            # contiguous in the last axis. So transpose the same dc for 4
            # consecutive kc into one PSUM group, then copy to
            # mht[:, dc, kc0*128:(kc0+4)*128].
            mh_nat = mres_pool.tile([P, NKC, D], f16)
            mht = mres_pool.tile([P, NDC, LKV], f16)
            mlt = mres_pool.tile([P, NDC, LKV], f16)
            for kc0 in range(0, NKC, 4):
                ml_chunks = {}
                for kc in range(kc0, kc0 + 4):
                    m_chunk = msplit_pool.tile(
                        [P, D], f32, tag="mchunk", bufs=6, name=f"mc{kc}"
                    )
                    nc.sync.dma_start(
                        out=m_chunk, in_=m_ap[kc * P:(kc + 1) * P, :]
                    )
                    # hi = fp16(M), lo = fp16(M - hi)
                    nc.vector.tensor_copy(mh_nat[:, kc, :], m_chunk)
                    ml_chunk = msplit_pool.tile(
                        [P, D], f16, tag="mlchunk", bufs=6, name=f"ml{kc}"
                    )
                    if kc % 2 == 0:
                        nc.gpsimd.tensor_sub(ml_chunk, m_chunk, mh_nat[:, kc, :])
                    else:
                        nc.vector.tensor_sub(ml_chunk, m_chunk, mh_nat[:, kc, :])
                    ml_chunks[kc] = ml_chunk
                for dc in range(NDC):
                    tg = ps_t.tile([P, 4 * P], f16, tag="tps16")
                    for j in range(4):
                        kc = kc0 + j
                        nc.tensor.transpose(
                            tg[:, j * P:(j + 1) * P],
                            mh_nat[:, kc, dc * P:(dc + 1) * P],
                            ident16,
                        )
                    if dc % 2 == 0:
                        nc.vector.tensor_copy(
                            mht[:, dc, kc0 * P:(kc0 + 4) * P], tg
                        )
                    else:
                        nc.scalar.copy(mht[:, dc, kc0 * P:(kc0 + 4) * P], tg)
                for dc in range(NDC):
                    tg = ps_t.tile([P, 4 * P], f16, tag="tps16")
                    for j in range(4):
                        kc = kc0 + j
                        nc.tensor.transpose(
                            tg[:, j * P:(j + 1) * P],
                            ml_chunks[kc][:, dc * P:(dc + 1) * P],
                            ident16,
                        )
                    if dc % 2 == 0:
                        nc.scalar.copy(mlt[:, dc, kc0 * P:(kc0 + 4) * P], tg)
                    else:
                        nc.vector.tensor_copy(
                            mlt[:, dc, kc0 * P:(kc0 + 4) * P], tg
                        )

            # ---- main loop over q tiles (Q-load/split/transpose of tile i+1
            # is emitted right after mm1(i) so the PE fills the softmax-latency
            # window with next-tile transposes).
            def load_qt(qt_i):
                """DMA q rows, scale by 50, split hi/lo fp16, PE-transpose
                into [d, q] layout."""
                q_nat = qload_pool.tile([P, D], f32, tag="qnat", name=f"qn{qt_i}")
                nc.sync.dma_start(
                    out=q_nat, in_=q_ap[qt_i * P:(qt_i + 1) * P, :]
                )
                qs = qsplit_pool.tile([P, D], f32, tag="qs", name=f"qs{qt_i}")
                nc.vector.tensor_scalar_mul(qs, q_nat, SCALE)
                qh = qsplit_pool.tile([P, D], f16, tag="qh", name=f"qh{qt_i}")
                nc.vector.tensor_copy(qh, qs)
                ql = qsplit_pool.tile([P, D], f16, tag="ql", name=f"ql{qt_i}")
                nc.vector.tensor_sub(ql, qs, qh)
                return qh, ql

            def transpose_qt(qsplit, qt_i):
                """Transpose the hi/lo Q into [d, q] layout. Either DMA xbar
                transposes (2-byte path, off the PE), or PE transposes with
                grouped PSUM tiles -> one [128,512] copy per 4 blocks."""
                qh, ql = qsplit
                qh_t = qt_pool.tile([P, NDC, P], f16, tag="qht", name=f"qht{qt_i}")
                ql_t = qt_pool.tile([P, NDC, P], f16, tag="qlt", name=f"qlt{qt_i}")
                if QT_DMA_T:
                    for src, dst in ((qh, qh_t), (ql, ql_t)):
                        for dc in range(NDC):
                            nc.sync.dma_start(
                                out=dst[:, dc, :],
                                in_=src[:, dc * P:(dc + 1) * P],
                                transpose=True,
                            )
                    return qh_t, ql_t
                for src, dst, eng in ((qh, qh_t, 0), (ql, ql_t, 1)):
                    for dc0 in range(0, NDC, 4):
                        tg = ps_t.tile([P, 4 * P], f16, tag="tps16")
                        for j in range(4):
                            dc = dc0 + j
                            nc.tensor.transpose(
                                tg[:, j * P:(j + 1) * P],
                                src[:, dc * P:(dc + 1) * P],
                                ident16,
                            )
                        if (eng + dc0 // 4) % 2 == 0:
                            nc.vector.tensor_copy(dst[:, dc0:dc0 + 4, :], tg)
                        else:
                            nc.scalar.copy(dst[:, dc0:dc0 + 4, :], tg)
                return qh_t, ql_t

            loop_cm = (
                tc.For_i(0, loop_r, 1) if loop_r else contextlib.nullcontext()
            )
            with loop_cm:
                qs_next = load_qt(0)
                qt_next = transpose_qt(qs_next, 0)
                for qt_i in range(NQT):
                    qh_t, ql_t = qt_next
                    # prefetch + split of tile i+1 runs on DVE during mm1(i)
                    if qt_i + 1 < NQT:
                        qs_next = load_qt(qt_i + 1)

                    # mm1: S[q, k] = (qh+ql) @ (mh+ml)^T via 3 fp16 passes,
                    # accumulated in PSUM. One PSUM tile per bank so the
                    # per-bank reduce_max starts as soon as that bank's
                    # accumulation closes (overlapping the rest of mm1).
                    s_banks = [
                        ps_s.tile([P, 512], f32, tag=f"s{ns}", name=f"s{ns}")
                        for ns in range(NS1)
                    ]
                    rowmax4 = vec_pool.tile([P, NS1], f32, tag="rm4")
                    if MM1_PAIRED:
                        # bank pairs share each loaded stationary operand
                        for ns0 in range(0, NS1, 2):
                            n_mm = 3 * NDC
                            i_mm = 0
                            for lhsT, rhs in (
                                (qh_t, mht), (qh_t, mlt), (ql_t, mht)
                            ):
                                for dc in range(NDC):
                                    for ns in (ns0, ns0 + 1):
                                        sl = slice(ns * 512, (ns + 1) * 512)
                                        nc.tensor.matmul(
                                            s_banks[ns],
                                            lhsT=lhsT[:, dc, :],
                                            rhs=rhs[:, dc, sl],
                                            start=(i_mm == 0),
                                            stop=(i_mm == n_mm - 1),
                                        )
                                    i_mm += 1
                            for ns in (ns0, ns0 + 1):
                                nc.vector.reduce_max(
                                    out=rowmax4[:, ns:ns + 1],
                                    in_=s_banks[ns],
                                    axis=AX,
                                )
                    else:
                        for ns in range(NS1):
                            sl = slice(ns * 512, (ns + 1) * 512)
                            n_mm = 3 * NDC
                            i_mm = 0
                            for lhsT, rhs in (
                                (qh_t, mht), (qh_t, mlt), (ql_t, mht)
                            ):
                                for dc in range(NDC):
                                    nc.tensor.matmul(
                                        s_banks[ns],
                                        lhsT=lhsT[:, dc, :],
                                        rhs=rhs[:, dc, sl],
                                        start=(i_mm == 0),
                                        stop=(i_mm == n_mm - 1),
                                    )
                                    i_mm += 1
                            # per-bank row max overlaps remaining mm1 banks
                            nc.vector.reduce_max(
                                out=rowmax4[:, ns:ns + 1], in_=s_banks[ns], axis=AX
                            )

                    # next tile's Q transposes: fill the PE gap while the softmax
                    # chain (last reduce + exp) runs.
                    if qt_i + 1 < NQT:
                        qt_next = transpose_qt(qs_next, qt_i + 1)

                    # S is already scaled by 50 (Q was), so bias is just -rowmax.
                    nbias = vec_pool.tile([P, 1], f32, tag="nbias")
                    nc.vector.reduce_max(out=nbias, in_=rowmax4, axis=AX, negate=True)

                    p_sb = p_pool.tile([P, LKV], f16, tag="p")
                    sums4 = vec_pool.tile([P, NS1], f32, tag="sm4")
                    for ns in range(NS1):
                        nc.scalar.activation(
                            p_sb[:, ns * 512:(ns + 1) * 512],
                            s_banks[ns],
                            EXP,
                            bias=nbias,
                            scale=1.0,
                            accum_out=sums4[:, ns:ns + 1],
                        )
                    sums = vec_pool.tile([P, 1], f32, tag="sm")
                    rsum = vec_pool.tile([P, 1], f32, tag="rs")
                    nc.vector.reduce_sum(out=sums, in_=sums4, axis=AX)
                    nc.vector.reciprocal(rsum, sums)

                    # P^T tiles (grouped copies) + mm2 (fp16)
                    pt_t = pt_pool.tile([P, NKC, P], f16, tag="pt")
                    o_psum = ps_o.tile([P, D], f32, tag="o")
                    for kc0 in range(0, NKC, 4):
                        tg = ps_t.tile([P, 4 * P], f16, tag="tps16")
                        for j in range(4):
                            kc = kc0 + j
                            nc.tensor.transpose(
                                tg[:, j * P:(j + 1) * P],
                                p_sb[:, kc * P:(kc + 1) * P],
                                ident16,
                            )
                        if (kc0 // 4) % 2 == 0:
                            nc.vector.tensor_copy(pt_t[:, kc0:kc0 + 4, :], tg)
                        else:
                            nc.scalar.copy(pt_t[:, kc0:kc0 + 4, :], tg)
                    # ns outer: 16 consecutive matmuls per PSUM bank — avoids
                    # per-instruction bank alternation (HAM depth-cycling).
                    for ns in range(NS2):
                        for kc in range(NKC):
                            nc.tensor.matmul(
                                o_psum[:, ns * 512:(ns + 1) * 512],
                                lhsT=pt_t[:, kc, :],
                                rhs=mh_nat[:, kc, ns * 512:(ns + 1) * 512],
                                start=(kc == 0),
                                stop=(kc == NKC - 1),
                            )

                    # scale by 1/rowsum on ACT (Copy with per-partition scale),
                    # keeping DVE free for the transpose copies.
                    out_sb = out_pool.tile([P, D], f32, tag="ot")
                    nc.scalar.activation(
                        out_sb, o_psum, mybir.ActivationFunctionType.Copy,
                        bias=0.0, scale=rsum,
                    )
                    nc.sync.dma_start(
                        out=o_ap[qt_i * P:(qt_i + 1) * P, :], in_=out_sb
                    )

    return _patch_json(nc)


def get_nc():
    if "nc" not in _CACHE:
        _CACHE["nc"] = build_nc()
    return _CACHE["nc"]


def kernel(query, memory):
    from concourse.bass_utils import run_bass_kernel_spmd

    q = np.ascontiguousarray(np.asarray(query, dtype=np.float32))
    m = np.ascontiguousarray(np.asarray(memory, dtype=np.float32))
    assert q.shape == (B, LQ, D) and m.shape == (B, LKV, D)

    nc = get_nc()
    in_maps = [{"q": q[b], "m": m[b]} for b in range(B)]
    res = run_bass_kernel_spmd(nc, in_maps, core_ids=list(range(B)))
    out = np.stack([res.results[b]["out"] for b in range(B)], axis=0)
    return out



# revision 26
# speedup vs baseline: 11.2259x; 11.2259x over previous
"""Trainium2 Bass kernel for batched attention:
    out[b] = softmax(q[b] @ m[b].T / 0.02) @ m[b]
with q, m: [8, 2048, 1024] fp32.

Sharding: data-parallel over batch — core b computes batch element b.

Numerics: the softmax temperature (x50) makes logits huge (std ~1600), so
the scores matmul needs ~19 bits of relative precision to keep the
near-one-hot softmax stable. mm1 runs as a SINGLE float32r pass: the PE
reads fp32 operands truncated to fp22 (e8m13) at 1 cycle/row for N>=512 —
same speed as fp16 but ~14-bit mantissa. Offline simulation vs the fp32
reference gives L2 1.7e-3 (gate: 2e-2); the fp16 3-pass hi/lo scheme this
replaces measured 2.1e-4 but cost 3x the PE time. The x50 scale is folded
into the exp activation (exp(50*S - 50*max)), so Q needs no preprocessing
at all. mm2 (P @ M) is plain fp16: P's dominant weights are exactly
representable and fp16(M)'s 11-bit mantissa bounds the output error at
~2e-4, dwarfed by the mm1 term.

Per-core dataflow (Lq=Lkv=2048, D=1024, q-tile = 128 rows):
  setup:  m_nat f32 staged by k-chunk ->
          mh16  [128,16,1024] f16  = fp16(M)   (mm2 moving operand)
          mtr   [128,8,2048]  f32  = M^T by d-chunk (PE f32r transposes)
  per q-tile:
          qt    [128,8,128]   f32  = Q_tile^T (PE f32r transposes)
          S     = qt.T @ mtr (f32r) -> PSUM [128, 4x512] f32, 1 pass
          P     = exp(50*S - 50*rowmax) -> SBUF f16 (ACT; accum_out = sums)
          PT    [128,16,128]  f16  = P^T (PE transposes)
          O     = PT.T @ mh16 -> PSUM [128,1024] f32
          out   = O * (1/rowsum) -> f32 -> DMA out
"""

import sys

if "/opt/trn_rl_repo" not in sys.path:
    sys.path.insert(0, "/opt/trn_rl_repo")

import os

import numpy as np

# Wait-split carrier opcode: "Drain" (safe: waits + pipe-flush) or "NoOp".
SPLIT_OPCODE = os.environ.get("ATTN_SPLIT_OPCODE", "NoOp")
# Transpose dtype for the f32 transposes: "f32r" (1.5 cyc/row) or "f32"
# (2 cyc/row).
TPOSE_DT = os.environ.get("ATTN_TPOSE_DT", "f32")
# Q stationary dtype for mm1: "f16" (fast FWL weight loads, ~11-bit q) or
# "f32r" (self-loading 4-byte weight loads, ~11-bit truncated q).
Q16 = os.environ.get("ATTN_Q16", "0") == "1"

B = 8
LQ = 2048
LKV = 2048
D = 1024
P = 128
NQT = LQ // P       # 16 q tiles
NKC = LKV // P      # 16 k chunks
NDC = D // P        # 8 d chunks
NS1 = LKV // 512    # 4 n-slices for mm1 (one PSUM bank each)
NS2 = D // 512      # 2 n-slices for mm2
SCALE = 1.0 / 0.02  # 50.0

_CACHE = {}


def _patch_json(nc):
    """This container's walrus supports only ONE sync-wait per instruction.
    Split any multi-wait instruction into preceding single-wait Drains on
    the same engine (engines execute in order, so semantics are identical)."""
    import orjson

    orig = nc.to_json_bytes

    def fixed():
        d = orjson.loads(orig())
        for fn in d["functions"]:
            for bb in fn["blocks"]:
                new = []
                for inst in bb.get("instructions", []):
                    si = inst.get("sync_info") or {}
                    ow = si.get("on_wait") or []
                    if len(ow) > 1:
                        excess, keep = ow[:-1], ow[-1:]
                        si["on_wait"] = keep
                        for k, w in enumerate(excess):
                            new.append({
                                "debug": inst.get("debug", 0),
                                "engine": inst["engine"],
                                "ins": [], "outs": [],
                                "is_reset_sema": False,
                                "name": f"{inst['name']}-sw{k}",
                                "opcode": SPLIT_OPCODE,
                                "sync_info": {"on_update": [], "on_wait": [w]},
                            })
                    new.append(inst)
                bb["instructions"] = new
        return orjson.dumps(d)

    nc.to_json_bytes = fixed
    return nc


def build_nc(loop_r=None):
    """loop_r: when set, wrap the main q-tile loop in a hardware For_i that
    repeats it loop_r times — used only for device-time measurement."""
    import contextlib

    import concourse.bass as bass
    import concourse.mybir as mybir
    import concourse.tile as tile
    from concourse.masks import make_identity

    f32 = mybir.dt.float32
    f32r = mybir.dt.float32r
    f16 = mybir.dt.float16
    AX = mybir.AxisListType.X
    EXP = mybir.ActivationFunctionType.Exp

    tdt = f32r if TPOSE_DT == "f32r" else f32

    def r(ap):
        """view an f32 AP as float32r (bit-identical)"""
        return ap.bitcast(f32r)

    def t(ap):
        return ap.bitcast(tdt) if TPOSE_DT == "f32r" else ap

    nc = bass.Bass()
    q_d = nc.dram_tensor("q", [LQ, D], f32, kind="ExternalInput")
    m_d = nc.dram_tensor("m", [LKV, D], f32, kind="ExternalInput")
    o_d = nc.dram_tensor("out", [LQ, D], f32, kind="ExternalOutput")

    q_ap = q_d.ap()
    m_ap = m_d.ap()
    o_ap = o_d.ap()

    with tile.TileContext(nc) as tc:
        with (
            tc.tile_pool(name="const", bufs=1) as const_pool,
            tc.tile_pool(name="mres", bufs=1) as mres_pool,
            tc.tile_pool(name="qload", bufs=3) as qload_pool,
            tc.tile_pool(name="qt", bufs=2) as qt_pool,
            tc.tile_pool(name="psb", bufs=2) as p_pool,
            tc.tile_pool(name="ptt", bufs=2) as pt_pool,
            tc.tile_pool(name="osb", bufs=3) as out_pool,
            tc.tile_pool(name="vec", bufs=6) as vec_pool,
            tc.tile_pool(name="msplit", bufs=4) as msplit_pool,
            tc.tile_pool(name="ps_s", bufs=1, space="PSUM") as ps_s,
            tc.tile_pool(name="ps_o", bufs=1, space="PSUM") as ps_o,
            tc.tile_pool(name="ps_t", bufs=2, space="PSUM") as ps_t,
        ):
            ident16 = const_pool.tile([P, P], f16)
            make_identity(nc, ident16)
            ident32 = const_pool.tile([P, P], f32)
            make_identity(nc, ident32)

            # ---- resident M derivatives: mh16 (f16, natural) for mm2 and
            # mtr (f32, transposed by d-chunk) for mm1.
            # Transposes land in grouped [128, 4x128] f32 PSUM tiles (one
            # bank) so ONE [128,512] copy moves four transposed blocks.
            # Four consecutive k-chunks of the same d-chunk share a group:
            # dest mtr[:, dc, kc0*128:(kc0+4)*128] is contiguous.
            mh16 = mres_pool.tile([P, NKC, D], f16)
            mtr = mres_pool.tile([P, NDC, LKV], f32r)

            def tpose_group4(srcs, dst, alt, eng=None):
                """PE-transpose four [128,128] f32 blocks into one [128,512]
                f32 PSUM tile (2KB = one bank), then one copy to dst."""
                tg = ps_t.tile([P, 4 * P], f32, tag="tps32")
                for j, s in enumerate(srcs):
                    nc.tensor.transpose(
                        t(tg[:, j * P:(j + 1) * P]), t(s), t(ident32)
                    )
                if eng == "act" or (eng is None and alt % 2 == 1):
                    nc.scalar.copy(dst, tg)
                else:
                    nc.vector.tensor_copy(dst, tg)

            # ---- main loop over q tiles (Q-load/transpose of tile i+1 is
            # emitted right after mm1(i) so the PE fills the softmax-latency
            # window with next-tile transposes).
            def load_qt(qt_i):
                q_nat = qload_pool.tile([P, D], f32, tag="qnat", name=f"qn{qt_i}")
                # Activation-engine HWDGE queue: q loads never wait behind
                # the 8MB M load / out stores on the SP queue.
                nc.scalar.dma_start(
                    out=q_nat, in_=q_ap[qt_i * P:(qt_i + 1) * P, :]
                )
                if not Q16:
                    return q_nat
                # fp16 is as precise as the ~11-bit f32r truncation on this
                # silicon and makes the mm1 stationary FWL-fast. No x50
                # pre-scale needed: fp16 is scale-invariant and the x50 is
                # folded into exp.
                q16 = qload_pool.tile([P, D], f16, tag="q16", name=f"q16_{qt_i}")
                nc.vector.tensor_copy(q16, q_nat)
                return q16

            def transpose_qt(q_nat, qt_i):
                """PE-transpose the Q tile into [d, q] layout with grouped
                PSUM staging."""
                if Q16:
                    # one [128, 8x128] f16 group = one PSUM bank, one copy
                    q_t = qt_pool.tile([P, NDC, P], f16, tag="qtt",
                                       name=f"qt{qt_i}")
                    tg32 = ps_t.tile([P, 4 * P], f32, tag="tps32", name="tgq")
                    tg = tg32.bitcast(f16)
                    for dc in range(NDC):
                        nc.tensor.transpose(
                            tg[:, dc * P:(dc + 1) * P],
                            q_nat[:, dc * P:(dc + 1) * P],
                            ident16,
                        )
                    nc.scalar.copy(q_t, tg)
                    return q_t
                q_t = qt_pool.tile([P, NDC, P], f32r, tag="qtt", name=f"qt{qt_i}")
                for g in range(NDC // 4):
                    da = 4 * g
                    tpose_group4(
                        [q_nat[:, (da + j) * P:(da + j + 1) * P]
                         for j in range(4)],
                        q_t[:, da:da + 4, :],
                        g, eng="act",
                    )
                return q_t

            loop_cm = (
                tc.For_i(0, loop_r, 1) if loop_r else contextlib.nullcontext()
            )

            def alloc_sbanks():
                s_banks = [
                    ps_s.tile([P, 512], f32, tag=f"s{ns}", name=f"s{ns}")
                    for ns in range(NS1)
                ]
                rowmax4 = vec_pool.tile([P, NS1], f32, tag="rm4")
                return s_banks, rowmax4

            def mm1_bank(q_t, s_banks, rowmax4, ns):
                """One PSUM bank of S = Q @ M^T (f32r single pass) + its
                row max (overlaps the remaining banks' matmuls)."""
                sl = slice(ns * 512, (ns + 1) * 512)
                for dc in range(NDC):
                    nc.tensor.matmul(
                        s_banks[ns],
                        lhsT=q_t[:, dc, :],
                        rhs=mtr[:, dc, sl],
                        start=(dc == 0),
                        stop=(dc == NDC - 1),
                    )
                nc.vector.reduce_max(
                    out=rowmax4[:, ns:ns + 1], in_=s_banks[ns], axis=AX
                )

            # ---- M prep: DMA chunks, f16 convert, PE-transpose into mtr.
            # Prep group g (kc 4g..4g+3) produces exactly the mtr columns
            # mm1 bank g streams, so outside the timing loop tile 0's mm1
            # bank g is emitted right after group g — hiding tile 0's mm1
            # under the DMA-bound prep.
            interleave0 = loop_r is None
            qn0 = load_qt(0)
            qt0 = transpose_qt(qn0, 0)
            if interleave0:
                sb0 = alloc_sbanks()
            for g, kc0 in enumerate(range(0, NKC, 4)):
                chunks = {}
                for kc in range(kc0, kc0 + 4):
                    m_chunk = msplit_pool.tile(
                        [P, D], f32, tag="mchunk", bufs=8, name=f"mc{kc}"
                    )
                    nc.sync.dma_start(
                        out=m_chunk, in_=m_ap[kc * P:(kc + 1) * P, :]
                    )
                    if kc % 2 == 0:
                        nc.vector.tensor_copy(mh16[:, kc, :], m_chunk)
                    else:
                        nc.gpsimd.tensor_copy(mh16[:, kc, :], m_chunk)
                    chunks[kc] = m_chunk
                for dc in range(NDC):
                    tpose_group4(
                        [chunks[kc0 + j][:, dc * P:(dc + 1) * P]
                         for j in range(4)],
                        mtr[:, dc, kc0 * P:(kc0 + 4) * P],
                        dc,
                    )
                if interleave0:
                    mm1_bank(qt0, sb0[0], sb0[1], g)

            def softmax(qt_i, s_banks, rowmax4):
                """exp(50*S - 50*rowmax) -> p_sb f16 + 1/rowsum. Returns
                (p_sb, rsum) consumed by tail() one tile later."""
                nmax = vec_pool.tile([P, 1], f32, tag="nmax")
                nc.vector.reduce_max(out=nmax, in_=rowmax4, axis=AX, negate=True)
                nbias = vec_pool.tile([P, 1], f32, tag="nbias")
                nc.vector.tensor_scalar_mul(nbias, nmax, SCALE)

                p_sb = p_pool.tile([P, LKV], f16, tag="p")
                sums4 = vec_pool.tile([P, NS1], f32, tag="sm4")
                for ns in range(NS1):
                    nc.scalar.activation(
                        p_sb[:, ns * 512:(ns + 1) * 512],
                        s_banks[ns],
                        EXP,
                        bias=nbias,
                        scale=SCALE,
                        accum_out=sums4[:, ns:ns + 1],
                    )
                return p_sb, sums4

            def tail_head(qt_i, p_sb, sums4):
                """P^T transposes + mm2 + out-scale + store for tile qt_i.
                Runs one tile delayed so exp(qt_i) has long completed and the
                PE never waits on the softmax chain. rm3_fn (the current
                tile's last-bank reduce_max) is emitted after the PT drains
                so those lead the DVE queue."""
                pt_t = pt_pool.tile([P, NKC, P], f16, tag="pt")
                o_psum = ps_o.tile([P, D], f32, tag="o")
                # P^T via ps_t staging banks; drains are half-bank DVE
                # copies so mm2 can start right after the first one.
                for g in range(2):
                    kc0 = 8 * g
                    tg32 = ps_t.tile([P, 4 * P], f32, tag="tps32", name="tgp")
                    tg = tg32.bitcast(f16)
                    for j in range(8):
                        kc = kc0 + j
                        nc.tensor.transpose(
                            tg[:, j * P:(j + 1) * P],
                            p_sb[:, kc * P:(kc + 1) * P],
                            ident16,
                        )
                    nc.vector.tensor_copy(pt_t[:, kc0:kc0 + 4, :],
                                          tg[:, 0:512])
                    nc.vector.tensor_copy(pt_t[:, kc0 + 4:kc0 + 8, :],
                                          tg[:, 512:1024])
                sums = vec_pool.tile([P, 1], f32, tag="sm")
                rsum = vec_pool.tile([P, 1], f32, tag="rs")
                nc.vector.reduce_sum(out=sums, in_=sums4, axis=AX)
                nc.vector.reciprocal(rsum, sums)
                # ns outer: 16 consecutive matmuls per PSUM bank — avoids
                # per-instruction bank alternation.
                for ns in range(NS2):
                    for kc in range(NKC):
                        nc.tensor.matmul(
                            o_psum[:, ns * 512:(ns + 1) * 512],
                            lhsT=pt_t[:, kc, :],
                            rhs=mh16[:, kc, ns * 512:(ns + 1) * 512],
                            start=(kc == 0),
                            stop=(kc == NKC - 1),
                        )

                return o_psum, rsum

            def tail_fin(qt_i, o_psum, rsum):
                # scale by 1/rowsum on ACT (Copy with per-partition scale);
                # emitted after softmax so exp leads the ACT queue.
                out_sb = out_pool.tile([P, D], f32, tag="ot")
                nc.scalar.activation(
                    out_sb, o_psum, mybir.ActivationFunctionType.Copy,
                    bias=0.0, scale=rsum,
                )
                nc.sync.dma_start(
                    out=o_ap[qt_i * P:(qt_i + 1) * P, :], in_=out_sb
                )

            with loop_cm:
                if interleave0:
                    qt_next = qt0
                else:
                    qn_next = load_qt(0)
                    qt_next = transpose_qt(qn_next, 0)
                prev = None  # (p_sb, sums4) of the previous tile
                for qt_i in range(NQT):
                    q_t = qt_next
                    # prefetch of tile i+1 runs during mm1(i)
                    if qt_i + 1 < NQT:
                        qn_next = load_qt(qt_i + 1)

                    if qt_i == 0 and interleave0:
                        s_banks, rowmax4 = sb0  # mm1 ran during prep
                    else:
                        s_banks, rowmax4 = alloc_sbanks()
                        for ns in range(NS1):
                            mm1_bank(q_t, s_banks, rowmax4, ns)

                    # the next tile's Q transposes fill the softmax-latency
                    # window; the previous tile's PT/mm2 follows (its staging
                    # drains are ancient by then).
                    if qt_i + 1 < NQT:
                        qt_next = transpose_qt(qn_next, qt_i + 1)
                    cur = softmax(qt_i, s_banks, rowmax4)
                    if prev is not None:
                        fin = tail_head(qt_i - 1, *prev)
                        tail_fin(qt_i - 1, *fin)
                    prev = cur
                fin = tail_head(NQT - 1, *prev)
                tail_fin(NQT - 1, *fin)

    return _patch_json(nc)


def get_nc():
    if "nc" not in _CACHE:
        _CACHE["nc"] = build_nc()
    return _CACHE["nc"]


def kernel(query, memory):
    from concourse.bass_utils import run_bass_kernel_spmd

    q = np.ascontiguousarray(np.asarray(query, dtype=np.float32))
    m = np.ascontiguousarray(np.asarray(memory, dtype=np.float32))
    assert q.shape == (B, LQ, D) and m.shape == (B, LKV, D)

    nc = get_nc()
    in_maps = [{"q": q[b], "m": m[b]} for b in range(B)]
    res = run_bass_kernel_spmd(nc, in_maps, core_ids=list(range(B)))
    out = np.stack([res.results[b]["out"] for b in range(B)], axis=0)
    return out


# revision 27
# speedup vs baseline: 11.5055x; 1.0249x over previous
"""Trainium2 Bass kernel for batched attention:
    out[b] = softmax(q[b] @ m[b].T / 0.02) @ m[b]
with q, m: [8, 2048, 1024] fp32.

Sharding: data-parallel over batch — core b computes batch element b.

Numerics: the softmax temperature (x50) makes logits huge (std ~1600), so
the scores matmul needs high relative precision to keep the near-one-hot
softmax stable. mm1 runs as a SINGLE float32r pass: the PE reads fp32
operands truncated to ~fp22 at 1 cycle/row for N>=256 — same streaming
speed as fp16 (the 4-byte self-loading weight loads cost ~107ns/matmul
extra; fp16/f32r operand mixing is rejected by the compiler, so this is
the floor). Measured end-to-end L2 vs the fp32 reference: 6.1e-3
(gate: 2e-2; the effective mantissa behaves like ~11 bits on silicon).
The fp16 3-pass hi/lo scheme this replaces measured 2.1e-4 but cost 3x
the PE time. The x50 scale is folded into the exp activation
(exp(50*S - 50*max)), so Q needs no preprocessing at all. mm2 (P @ M) is
plain fp16: P's dominant weights are exactly representable and fp16(M)'s
11-bit mantissa bounds its error contribution at ~2e-4.

Schedule (per-core, software-pipelined one tile deep):
  PE order: mm1(i) | Q^T(i+1) transposes | P^T(i-1) transposes | mm2(i-1).
  exp(i) + row-sum run on ACT/DVE under P^T/mm2(i-1); tile i's PT/mm2 run
  one tile later so the PE never waits on the softmax latency chain.
  Staging drains: Q^T groups -> ACT (ahead of exp in queue order), P^T
  groups -> DVE half-copies (mm2 starts after the first quarter).
  Outside the timing loop, tile 0's mm1 bank g is emitted right after M
  prep transpose group g, hiding it under the DMA-bound M load; q loads
  use the ACT HWDGE queue so they never queue behind the 8MB M load.

Per-core dataflow (Lq=Lkv=2048, D=1024, q-tile = 128 rows):
  setup:  m_nat f32 staged by k-chunk ->
          mh16  [128,16,1024] f16  = fp16(M)   (mm2 moving operand)
          mtr   [128,8,2048]  f32  = M^T by d-chunk (PE f32r transposes)
  per q-tile:
          qt    [128,8,128]   f32  = Q_tile^T (PE f32r transposes)
          S     = qt.T @ mtr (f32r) -> PSUM [128, 4x512] f32, 1 pass
          P     = exp(50*S - 50*rowmax) -> SBUF f16 (ACT; accum_out = sums)
          PT    [128,16,128]  f16  = P^T (PE transposes)
          O     = PT.T @ mh16 -> PSUM [128,1024] f32
          out   = O * (1/rowsum) -> f32 -> DMA out
"""

import sys

if "/opt/trn_rl_repo" not in sys.path:
    sys.path.insert(0, "/opt/trn_rl_repo")

import os

import numpy as np

# Wait-split carrier opcode: "Drain" (safe: waits + pipe-flush) or "NoOp".
SPLIT_OPCODE = os.environ.get("ATTN_SPLIT_OPCODE", "NoOp")
# Transpose dtype for the f32 transposes: "f32r" (1.5 cyc/row) or "f32"
# (2 cyc/row).
TPOSE_DT = os.environ.get("ATTN_TPOSE_DT", "f32")
# Q stationary dtype for mm1: "f16" (fast FWL weight loads, ~11-bit q) or
# "f32r" (self-loading 4-byte weight loads, ~11-bit truncated q).
Q16 = os.environ.get("ATTN_Q16", "0") == "1"

B = 8
LQ = 2048
LKV = 2048
D = 1024
P = 128
NQT = LQ // P       # 16 q tiles
NKC = LKV // P      # 16 k chunks
NDC = D // P        # 8 d chunks
NS1 = LKV // 512    # 4 n-slices for mm1 (one PSUM bank each)
NS2 = D // 512      # 2 n-slices for mm2
SCALE = 1.0 / 0.02  # 50.0

_CACHE = {}


def _patch_json(nc):
    """This container's walrus supports only ONE sync-wait per instruction.
    Split any multi-wait instruction into preceding single-wait Drains on
    the same engine (engines execute in order, so semantics are identical)."""
    import orjson

    orig = nc.to_json_bytes

    def fixed():
        d = orjson.loads(orig())
        for fn in d["functions"]:
            for bb in fn["blocks"]:
                new = []
                for inst in bb.get("instructions", []):
                    si = inst.get("sync_info") or {}
                    ow = si.get("on_wait") or []
                    if len(ow) > 1:
                        excess, keep = ow[:-1], ow[-1:]
                        si["on_wait"] = keep
                        for k, w in enumerate(excess):
                            new.append({
                                "debug": inst.get("debug", 0),
                                "engine": inst["engine"],
                                "ins": [], "outs": [],
                                "is_reset_sema": False,
                                "name": f"{inst['name']}-sw{k}",
                                "opcode": SPLIT_OPCODE,
                                "sync_info": {"on_update": [], "on_wait": [w]},
                            })
                    new.append(inst)
                bb["instructions"] = new
        return orjson.dumps(d)

    nc.to_json_bytes = fixed
    return nc


def build_nc(loop_r=None):
    """loop_r: when set, wrap the main q-tile loop in a hardware For_i that
    repeats it loop_r times — used only for device-time measurement."""
    import contextlib

    import concourse.bass as bass
    import concourse.mybir as mybir
    import concourse.tile as tile
    from concourse.masks import make_identity

    f32 = mybir.dt.float32
    f32r = mybir.dt.float32r
    f16 = mybir.dt.float16
    AX = mybir.AxisListType.X
    EXP = mybir.ActivationFunctionType.Exp

    tdt = f32r if TPOSE_DT == "f32r" else f32

    def r(ap):
        """view an f32 AP as float32r (bit-identical)"""
        return ap.bitcast(f32r)

    def t(ap):
        return ap.bitcast(tdt) if TPOSE_DT == "f32r" else ap

    nc = bass.Bass()
    q_d = nc.dram_tensor("q", [LQ, D], f32, kind="ExternalInput")
    m_d = nc.dram_tensor("m", [LKV, D], f32, kind="ExternalInput")
    o_d = nc.dram_tensor("out", [LQ, D], f32, kind="ExternalOutput")

    q_ap = q_d.ap()
    m_ap = m_d.ap()
    o_ap = o_d.ap()

    with tile.TileContext(nc) as tc:
        with (
            tc.tile_pool(name="const", bufs=1) as const_pool,
            tc.tile_pool(name="mres", bufs=1) as mres_pool,
            tc.tile_pool(name="qload", bufs=3) as qload_pool,
            tc.tile_pool(name="qt", bufs=2) as qt_pool,
            tc.tile_pool(name="psb", bufs=2) as p_pool,
            tc.tile_pool(name="ptt", bufs=2) as pt_pool,
            tc.tile_pool(name="osb", bufs=3) as out_pool,
            tc.tile_pool(name="vec", bufs=6) as vec_pool,
            tc.tile_pool(name="msplit", bufs=4) as msplit_pool,
            tc.tile_pool(name="ps_s", bufs=1, space="PSUM") as ps_s,
            tc.tile_pool(name="ps_o", bufs=1, space="PSUM") as ps_o,
            tc.tile_pool(name="ps_t", bufs=2, space="PSUM") as ps_t,
        ):
            ident16 = const_pool.tile([P, P], f16)
            make_identity(nc, ident16)
            ident32 = const_pool.tile([P, P], f32)
            make_identity(nc, ident32)

            # ---- resident M derivatives: mh16 (f16, natural) for mm2 and
            # mtr (f32, transposed by d-chunk) for mm1.
            # Transposes land in grouped [128, 4x128] f32 PSUM tiles (one
            # bank) so ONE [128,512] copy moves four transposed blocks.
            # Four consecutive k-chunks of the same d-chunk share a group:
            # dest mtr[:, dc, kc0*128:(kc0+4)*128] is contiguous.
            mh16 = mres_pool.tile([P, NKC, D], f16)
            mtr = mres_pool.tile([P, NDC, LKV], f32r)

            def tpose_group4(srcs, dst, alt, eng=None):
                """PE-transpose four [128,128] f32 blocks into one [128,512]
                f32 PSUM tile (2KB = one bank), then one copy to dst."""
                tg = ps_t.tile([P, 4 * P], f32, tag="tps32")
                for j, s in enumerate(srcs):
                    nc.tensor.transpose(
                        t(tg[:, j * P:(j + 1) * P]), t(s), t(ident32)
                    )
                if eng == "act" or (eng is None and alt % 2 == 1):
                    nc.scalar.copy(dst, tg)
                else:
                    nc.vector.tensor_copy(dst, tg)

            # ---- main loop over q tiles (Q-load/transpose of tile i+1 is
            # emitted right after mm1(i) so the PE fills the softmax-latency
            # window with next-tile transposes).
            def load_qt(qt_i):
                q_nat = qload_pool.tile([P, D], f32, tag="qnat", name=f"qn{qt_i}")
                # Activation-engine HWDGE queue: q loads never wait behind
                # the 8MB M load / out stores on the SP queue.
                nc.scalar.dma_start(
                    out=q_nat, in_=q_ap[qt_i * P:(qt_i + 1) * P, :]
                )
                if not Q16:
                    return q_nat
                # fp16 is as precise as the ~11-bit f32r truncation on this
                # silicon and makes the mm1 stationary FWL-fast. No x50
                # pre-scale needed: fp16 is scale-invariant and the x50 is
                # folded into exp.
                q16 = qload_pool.tile([P, D], f16, tag="q16", name=f"q16_{qt_i}")
                nc.vector.tensor_copy(q16, q_nat)
                return q16

            def transpose_qt(q_nat, qt_i):
                """PE-transpose the Q tile into [d, q] layout with grouped
                PSUM staging."""
                if Q16:
                    # one [128, 8x128] f16 group = one PSUM bank, one copy
                    q_t = qt_pool.tile([P, NDC, P], f16, tag="qtt",
                                       name=f"qt{qt_i}")
                    tg32 = ps_t.tile([P, 4 * P], f32, tag="tps32", name="tgq")
                    tg = tg32.bitcast(f16)
                    for dc in range(NDC):
                        nc.tensor.transpose(
                            tg[:, dc * P:(dc + 1) * P],
                            q_nat[:, dc * P:(dc + 1) * P],
                            ident16,
                        )
                    nc.scalar.copy(q_t, tg)
                    return q_t
                q_t = qt_pool.tile([P, NDC, P], f32r, tag="qtt", name=f"qt{qt_i}")
                for g in range(NDC // 4):
                    da = 4 * g
                    tpose_group4(
                        [q_nat[:, (da + j) * P:(da + j + 1) * P]
                         for j in range(4)],
                        q_t[:, da:da + 4, :],
                        g, eng="act",
                    )
                return q_t

            loop_cm = (
                tc.For_i(0, loop_r, 1) if loop_r else contextlib.nullcontext()
            )

            def alloc_sbanks():
                s_banks = [
                    ps_s.tile([P, 512], f32, tag=f"s{ns}", name=f"s{ns}")
                    for ns in range(NS1)
                ]
                rowmax4 = vec_pool.tile([P, NS1], f32, tag="rm4")
                return s_banks, rowmax4

            def mm1_bank(q_t, s_banks, rowmax4, ns):
                """One PSUM bank of S = Q @ M^T (f32r single pass) + its
                row max (overlaps the remaining banks' matmuls)."""
                sl = slice(ns * 512, (ns + 1) * 512)
                for dc in range(NDC):
                    nc.tensor.matmul(
                        s_banks[ns],
                        lhsT=q_t[:, dc, :],
                        rhs=mtr[:, dc, sl],
                        start=(dc == 0),
                        stop=(dc == NDC - 1),
                    )
                nc.vector.reduce_max(
                    out=rowmax4[:, ns:ns + 1], in_=s_banks[ns], axis=AX
                )

            # ---- M prep: DMA chunks, f16 convert, PE-transpose into mtr.
            # Prep group g (kc 4g..4g+3) produces exactly the mtr columns
            # mm1 bank g streams, so outside the timing loop tile 0's mm1
            # bank g is emitted right after group g — hiding tile 0's mm1
            # under the DMA-bound prep.
            interleave0 = loop_r is None
            qn0 = load_qt(0)
            qt0 = transpose_qt(qn0, 0)
            if interleave0:
                sb0 = alloc_sbanks()
            for g, kc0 in enumerate(range(0, NKC, 4)):
                chunks = {}
                for kc in range(kc0, kc0 + 4):
                    m_chunk = msplit_pool.tile(
                        [P, D], f32, tag="mchunk", bufs=8, name=f"mc{kc}"
                    )
                    nc.sync.dma_start(
                        out=m_chunk, in_=m_ap[kc * P:(kc + 1) * P, :]
                    )
                    if kc % 2 == 0:
                        nc.vector.tensor_copy(mh16[:, kc, :], m_chunk)
                    else:
                        nc.gpsimd.tensor_copy(mh16[:, kc, :], m_chunk)
                    chunks[kc] = m_chunk
                for dc in range(NDC):
                    tpose_group4(
                        [chunks[kc0 + j][:, dc * P:(dc + 1) * P]
                         for j in range(4)],
                        mtr[:, dc, kc0 * P:(kc0 + 4) * P],
                        dc,
                    )
                if interleave0:
                    mm1_bank(qt0, sb0[0], sb0[1], g)

            def softmax(qt_i, s_banks, rowmax4):
                """exp(50*S - 50*rowmax) -> p_sb f16 + 1/rowsum. Returns
                (p_sb, rsum) consumed by tail() one tile later."""
                nmax = vec_pool.tile([P, 1], f32, tag="nmax")
                nc.vector.reduce_max(out=nmax, in_=rowmax4, axis=AX, negate=True)
                nbias = vec_pool.tile([P, 1], f32, tag="nbias")
                nc.vector.tensor_scalar_mul(nbias, nmax, SCALE)

                p_sb = p_pool.tile([P, LKV], f16, tag="p")
                sums4 = vec_pool.tile([P, NS1], f32, tag="sm4")
                for ns in range(NS1):
                    nc.scalar.activation(
                        p_sb[:, ns * 512:(ns + 1) * 512],
                        s_banks[ns],
                        EXP,
                        bias=nbias,
                        scale=SCALE,
                        accum_out=sums4[:, ns:ns + 1],
                    )
                return p_sb, sums4

            def tail_head(qt_i, p_sb, sums4):
                """P^T transposes + mm2 + out-scale + store for tile qt_i.
                Runs one tile delayed so exp(qt_i) has long completed and the
                PE never waits on the softmax chain. rm3_fn (the current
                tile's last-bank reduce_max) is emitted after the PT drains
                so those lead the DVE queue."""
                pt_t = pt_pool.tile([P, NKC, P], f16, tag="pt")
                o_psum = ps_o.tile([P, D], f32, tag="o")
                # P^T via ps_t staging banks; drains are half-bank DVE
                # copies so mm2 can start right after the first one.
                for g in range(2):
                    kc0 = 8 * g
                    tg32 = ps_t.tile([P, 4 * P], f32, tag="tps32", name="tgp")
                    tg = tg32.bitcast(f16)
                    for j in range(8):
                        kc = kc0 + j
                        nc.tensor.transpose(
                            tg[:, j * P:(j + 1) * P],
                            p_sb[:, kc * P:(kc + 1) * P],
                            ident16,
                        )
                    nc.vector.tensor_copy(pt_t[:, kc0:kc0 + 4, :],
                                          tg[:, 0:512])
                    nc.vector.tensor_copy(pt_t[:, kc0 + 4:kc0 + 8, :],
                                          tg[:, 512:1024])
                sums = vec_pool.tile([P, 1], f32, tag="sm")
                rsum = vec_pool.tile([P, 1], f32, tag="rs")
                nc.vector.reduce_sum(out=sums, in_=sums4, axis=AX)
                nc.vector.reciprocal(rsum, sums)
                # ns outer: 16 consecutive matmuls per PSUM bank — avoids
                # per-instruction bank alternation.
                for ns in range(NS2):
                    for kc in range(NKC):
                        nc.tensor.matmul(
                            o_psum[:, ns * 512:(ns + 1) * 512],
                            lhsT=pt_t[:, kc, :],
                            rhs=mh16[:, kc, ns * 512:(ns + 1) * 512],
                            start=(kc == 0),
                            stop=(kc == NKC - 1),
                        )

                return o_psum, rsum

            def tail_fin(qt_i, o_psum, rsum):
                # scale by 1/rowsum on ACT (Copy with per-partition scale);
                # emitted after softmax so exp leads the ACT queue.
                out_sb = out_pool.tile([P, D], f32, tag="ot")
                nc.scalar.activation(
                    out_sb, o_psum, mybir.ActivationFunctionType.Copy,
                    bias=0.0, scale=rsum,
                )
                nc.sync.dma_start(
                    out=o_ap[qt_i * P:(qt_i + 1) * P, :], in_=out_sb
                )

            with loop_cm:
                if interleave0:
                    qt_next = qt0
                else:
                    qn_next = load_qt(0)
                    qt_next = transpose_qt(qn_next, 0)
                prev = None  # (p_sb, sums4) of the previous tile
                for qt_i in range(NQT):
                    q_t = qt_next
                    # prefetch of tile i+1 runs during mm1(i)
                    if qt_i + 1 < NQT:
                        qn_next = load_qt(qt_i + 1)

                    if qt_i == 0 and interleave0:
                        s_banks, rowmax4 = sb0  # mm1 ran during prep
                    else:
                        s_banks, rowmax4 = alloc_sbanks()
                        for ns in range(NS1):
                            mm1_bank(q_t, s_banks, rowmax4, ns)

                    # the next tile's Q transposes fill the softmax-latency
                    # window; the previous tile's PT/mm2 follows (its staging
                    # drains are ancient by then).
                    if qt_i + 1 < NQT:
                        qt_next = transpose_qt(qn_next, qt_i + 1)
                    cur = softmax(qt_i, s_banks, rowmax4)
                    if prev is not None:
                        fin = tail_head(qt_i - 1, *prev)
                        tail_fin(qt_i - 1, *fin)
                    prev = cur
                fin = tail_head(NQT - 1, *prev)
                tail_fin(NQT - 1, *fin)

    return _patch_json(nc)


def get_nc():
    if "nc" not in _CACHE:
        _CACHE["nc"] = build_nc()
    return _CACHE["nc"]


def kernel(query, memory):
    from concourse.bass_utils import run_bass_kernel_spmd

    q = np.ascontiguousarray(np.asarray(query, dtype=np.float32))
    m = np.ascontiguousarray(np.asarray(memory, dtype=np.float32))
    assert q.shape == (B, LQ, D) and m.shape == (B, LKV, D)

    nc = get_nc()
    in_maps = [{"q": q[b], "m": m[b]} for b in range(B)]
    res = run_bass_kernel_spmd(nc, in_maps, core_ids=list(range(B)))
    out = np.stack([res.results[b]["out"] for b in range(B)], axis=0)
    return out
